# revision 50
# baseline (speedup 1.0000x reference)
"""Trainium2 Bass kernel v7 for nn_MultiHeadAttention_86079734546451.

Sharding: data-parallel over batch B=16 across 8 cores (2 batches/core).
All weights replicated. No collectives. HW exec ~275us (v2 was 417us).

Design:
 - q/k/v host-transposed to [B, D, S, N] (s-major tokens): every per-s
   slice in scores/V-proj/AV is contiguous (v2 paid ~3x on strided
   s::S LDWEIGHTS/rhs streaming).
 - bias kron packs BOTH batches in one matmul: contraction (j,b2,l)=96
   partitions, block-diag wdk2 outputs (j,b2,s:16) partitions. Halves
   kron MMs, psum-evac copies, and transposes.
 - psum->sbuf kron copy IS the exp: softmax factorized as
   exp(scores+bias) = exp(scores)*exp(bias); ebias moved to m-partition
   layout by a DVE 32x32 stream transpose on an int32 view (n2 pair
   rides inside each i32; the transpose OUT AP scatters (b2,s) outward)
   so each per-(b,h,s) bias slice is a contiguous [128,128].
 - scores exp: ACT reads psum directly per s-quad; the bias multiply is
   a DVE scalar_tensor_tensor per (s,h2) with accum_out -> softmax z
   falls out free (no tensor_reduce, no psum tensor_add).
 - bv/bo folded into psum evacuation (DVE tensor_add with partition-
   replicated bias tiles); output staged bf16; no ones-matmuls.
 - driver: kron||P1(b0) (kron drained before P3's first DVE read of
   tb), P3(b0)||P1(b1) with b1's QK-co_k emission gated on b0's hp_k
   scores (per-name bufs=1 qkh tags make b1 reuse b0's tiles in exact
   release order -> acyclic engine queues), P4(b0)||P3(b1).
 - ebp bufs=5 lets the exp/mul/z chain run a full s-quad ahead of the
   deferred AV matmuls (biggest single win: ~30us).
"""

import sys

sys.path.insert(0, "/opt/trn_rl_repo")

from contextlib import ExitStack

import numpy as np

import concourse.bass as bass
import concourse.mybir as mybir
import concourse.tile as tile
from concourse import bacc

f32 = mybir.dt.float32
bf16 = mybir.dt.bfloat16
i32 = mybir.dt.int32
AF = mybir.ActivationFunctionType
ALU = mybir.AluOpType
AX = mybir.AxisListType

# Problem constants
B_LOC = 2          # batches per core
D = 512
N = 128            # nodes
S = 14             # seq
L = 12
H = 8
DH = 64            # head dim
TOK = N * S        # 1792 tokens per batch, (s, n) order
C = 4              # 128-chunks of D
NCORES = 8

QUADS = [(0, 4), (4, 8), (8, 12), (12, 14)]  # s-blocks


def emit_kernel(ctx: ExitStack, tc: "tile.TileContext", io: dict):
    nc = tc.nc

    q_d, k_d, v_d, ab_d = io["q"], io["k"], io["v"], io["ab"]
    out_d = io["out"]

    # ---------------- pools ----------------
    wpool = ctx.enter_context(tc.tile_pool(name="wpool", bufs=1))
    tbp = ctx.enter_context(tc.tile_pool(name="tbp", bufs=1))
    bsbp = ctx.enter_context(tc.tile_pool(name="bsbp", bufs=2))
    abp = ctx.enter_context(tc.tile_pool(name="abp", bufs=1))
    xin = ctx.enter_context(tc.tile_pool(name="xin", bufs=8))
    qkh = ctx.enter_context(tc.tile_pool(name="qkh", bufs=1))
    vhp = ctx.enter_context(tc.tile_pool(name="vhp", bufs=2))
    ebp = ctx.enter_context(tc.tile_pool(name="ebp", bufs=5))
    ytp = ctx.enter_context(tc.tile_pool(name="ytp", bufs=4))
    vpp = ctx.enter_context(tc.tile_pool(name="vpp", bufs=2))
    zrp = ctx.enter_context(tc.tile_pool(name="zrp", bufs=1))
    osbp = ctx.enter_context(tc.tile_pool(name="osbp", bufs=2))

    pb = ctx.enter_context(tc.tile_pool(name="pb", bufs=2, space="PSUM"))
    pq = ctx.enter_context(tc.tile_pool(name="pq", bufs=2, space="PSUM"))
    scp = ctx.enter_context(tc.tile_pool(name="scp", bufs=2, space="PSUM"))

    # ---------------- weights (once) ----------------
    wq, wk, wv, wo = [], [], [], []
    for c in range(C):
        for dst, nm in ((wq, "wqT"), (wk, "wkT"), (wv, "wvT"), (wo, "woT")):
            t = wpool.tile([128, D], bf16, name=f"{nm}{c}", tag=f"{nm}{c}")
            nc.scalar.dma_start(t[:], io[nm][c * 128:(c + 1) * 128, :])
            dst.append(t)

    wdk2 = wpool.tile([96, 128], bf16, name="wdk2", tag="wdk2")
    nc.scalar.dma_start(wdk2[:], io["wdk2"][:])

    bk_t = wpool.tile([128, C], f32, name="bk_t", tag="bk_t")
    for c in range(C):
        nc.scalar.dma_start(bk_t[:, c:c + 1],
                            io["bk"][c * 128:(c + 1) * 128].unsqueeze(1))

    bv_t = wpool.tile([128, D], bf16, name="bv_t", tag="bv_t")
    nc.gpsimd.dma_start(bv_t[:], io["bv2"][:])
    bo_t = wpool.tile([128, D], bf16, name="bo_t", tag="bo_t")
    nc.gpsimd.dma_start(bo_t[:], io["bo2"][:])
    ones_b = wpool.tile([1, 128], bf16, name="ones_b", tag="ones_b")
    nc.vector.memset(ones_b[:], 1.0)

    # ---------------- shared bias tile (both batches) ----------------
    # tb free layout: (b2:2, s:16, hq:4, h2:2, ch:16, nq:4, n2:2), so the
    # per-(b,hq,h2,s) bias slice [128, n:128] is fully contiguous (the
    # stream-transpose output AP scatters (b2,s) outward; the n2 pair
    # rides inside each int32 element)
    tb = tbp.tile([128, 16 * 2048], bf16, name="tb", tag="tb")
    tb32v = tb[:].bitcast(i32).rearrange(
        "p (b2 s hq h2 ch nq) -> p ch hq h2 nq (b2 s)",
        b2=2, s=16, hq=4, h2=2, ch=16, nq=4)
    tbv = tb[:].rearrange(
        "p (b2 s hq h2 r) -> p b2 hq h2 s r",
        b2=2, s=16, hq=4, h2=2)

    def kron_gen():
        """ebias = exp(Wd-projected attn_bias), transposed to m-partition
        layout, both batches at once."""
        for quar in range(4):
            for half in range(2):
                abt = abp.tile([96, 4096], bf16, tag="abt", name="abt")
                nc.sync.dma_start(abt[:], ab_d[quar, half])
                abtv = abt[:].rearrange(
                    "p (c2 hq r) -> p c2 hq r", c2=2, hq=4)
                for chl in range(2):
                    ch = quar * 4 + half * 2 + chl
                    bsb = bsbp.tile([128, 2048], bf16, tag="bsb", name="bsb")
                    for t in range(2):
                        pbt = pb.tile([128, 1024], f32, tag="pb", name="pbt")
                        for hq2 in range(2):
                            hq = t * 2 + hq2
                            nc.tensor.matmul(
                                pbt[:, hq2 * 512:(hq2 + 1) * 512],
                                lhsT=wdk2[:],
                                rhs=abtv[:, chl, hq, :],
                                start=True, stop=True)
                        nc.scalar.activation(
                            bsb[:, t * 1024:(t + 1) * 1024], pbt[:], AF.Exp)
                        yield "kron"
                    nc.vector.transpose(tb32v[:, ch], bsb[:].bitcast(i32))
                    yield "kron"

    # ---------------- per-batch body ----------------
    def batch_work(b):
        def load_x(src_d):
            xs = []
            for ci in range(C):
                x_c = xin.tile([128, TOK], bf16, tag="xin", name=f"x{ci}")
                nc.gpsimd.dma_start(
                    x_c[:],
                    src_d[b, ci * 128:(ci + 1) * 128].rearrange(
                        "p s n -> p (s n)"))
                xs.append(x_c)
            return xs

        def p1_co(xs, wts, co, dst_list, with_bias, tg):
            # per-name tag, bufs=1: b1's gen reuses exactly b0's slot,
            # which frees when b0's P3 hp=co scores have read it
            h_c = qkh.tile([128, TOK], bf16, tag=f"h{tg}{co}", name=f"h{tg}{co}")
            for tbk in range(4):
                ps = pq.tile([128, 512], f32, tag="pq", name="ps_qk")
                for ci in range(C):
                    nc.tensor.matmul(
                        ps[:, :448],
                        lhsT=wts[ci][:, co * 128:(co + 1) * 128],
                        rhs=xs[ci][:, tbk * 448:(tbk + 1) * 448],
                        start=(ci == 0), stop=(ci == C - 1))
                if with_bias:
                    nc.scalar.activation(h_c[:, tbk * 448:(tbk + 1) * 448],
                                         ps[:, :448], AF.Identity,
                                         bias=bk_t[:, co:co + 1], scale=1.0)
                elif b == 0:
                    # b0's Q copies on DVE (its P1 phase is ACT-bound:
                    # kron exp runs there)
                    nc.vector.tensor_copy(h_c[:, tbk * 448:(tbk + 1) * 448],
                                          ps[:, :448])
                else:
                    # b1's P1 overlaps P3(b0), where DVE is the binding
                    # engine (stt) and ACT has slack
                    nc.scalar.activation(h_c[:, tbk * 448:(tbk + 1) * 448],
                                         ps[:, :448], AF.Identity, scale=1.0)
            dst_list.append(h_c)

        # ---- P1 ----
        qh, kh = [], []
        yield "p1"
        xq = load_x(q_d)
        xk = load_x(k_d)
        for co in range(C):
            # gate marker: for b1 the driver holds here until b0's P3
            # hp=co scores are emitted (qkh slot release order)
            yield f"p1g{co}"
            p1_co(xq, wq, co, qh, False, "q")
            p1_co(xk, wk, co, kh, True, "k")
        yield "p1v"
        xv = load_x(v_d)
        vh = vhp.tile([128, S * D], bf16, tag="vh", name="vh")
        for s in range(S):
            ps = pq.tile([128, D], f32, tag="pq", name="ps_v")
            for ci in range(C):
                nc.tensor.matmul(
                    ps[:],
                    lhsT=xv[ci][:, s * N:(s + 1) * N],
                    rhs=wv[ci][:],
                    start=(ci == 0), stop=(ci == C - 1 and b == 0))
            if b == 0:
                nc.vector.tensor_add(vh[:, s * D:(s + 1) * D], ps[:], bv_t[:])
            else:
                # b1's V-evac off DVE: bv via ones-matmul, copy on ACT
                nc.tensor.matmul(ps[:], lhsT=ones_b[:], rhs=bv_t[0:1, :],
                                 start=False, stop=True)
                nc.scalar.copy(vh[:, s * D:(s + 1) * D], ps[:])
            if s % 2:
                yield "p1v"

        # ---- P3 ----
        z_t = zrp.tile([128, 128], f32, tag="z", name="z_t")
        rt_t = zrp.tile([128, 128], f32, tag="r", name="rt_t")
        zv = z_t[:].rearrange("p (h s) -> p h s", h=H)
        rv = rt_t[:].rearrange("p (h s) -> p h s", h=H)
        rvT = rt_t[:].rearrange("p (h s) -> p s h", h=H)
        vhv = vh[:].rearrange("p (s d) -> p s d", s=S)
        yt = [ytp.tile([128, TOK], bf16, tag="ytp", name=f"yt{c}")
              for c in range(C)]

        pend = [None]  # deferred AV work (one-quad software pipeline)

        def do_av(hp, qi, vpv, ex):
            s0, s1 = QUADS[qi]
            ls = s1 - s0
            av = pb.tile([128, 512], f32, tag="pb", name="av_t")
            for si in range(ls):
                for h2 in range(2):
                    nc.tensor.matmul(
                        av[h2 * DH:(h2 + 1) * DH, si * 128:(si + 1) * 128],
                        lhsT=vpv[:, si, h2, :],
                        rhs=ex[h2][:, si * 128:(si + 1) * 128],
                        start=True, stop=True)
            nc.scalar.copy(yt[hp][:, s0 * 128:s1 * 128], av[:, :ls * 128])

        for hp in range(C):
            for qi, (s0, s1) in enumerate(QUADS):
                ls = s1 - s0
                # scores for this quad
                sct = [scp.tile([128, 512], f32, tag="sc", name=f"sc{h2}")
                       for h2 in range(2)]
                for si in range(ls):
                    s = s0 + si
                    for h2 in range(2):
                        hb = h2 * DH
                        nc.tensor.matmul(
                            sct[h2][:, si * 128:(si + 1) * 128],
                            lhsT=kh[hp][hb:hb + DH, s * N:(s + 1) * N],
                            rhs=qh[hp][hb:hb + DH, s * N:(s + 1) * N],
                            start=True, stop=True)
                yield "p3"
                # deferred AV of previous quad (PE work lands after the
                # interleaved P1 units gave the chain time to finish)
                if pend[0] is not None:
                    do_av(*pend[0])
                    pend[0] = None
                # exp(scores) from psum; bias-mul with accum -> z
                ex = [ebp.tile([128, 512], bf16, tag="eb", name=f"eb{h2}")
                      for h2 in range(2)]
                for h2 in range(2):
                    nc.scalar.activation(ex[h2][:, :ls * 128],
                                         sct[h2][:, :ls * 128], AF.Exp)
                for h2 in range(2):
                    h = hp * 2 + h2
                    for si in range(ls):
                        s = s0 + si
                        esl = ex[h2][:, si * 128:(si + 1) * 128]
                        nc.vector.scalar_tensor_tensor(
                            esl, esl, 1.0, tbv[:, b, hp, h2, s],
                            op0=ALU.mult, op1=ALU.mult,
                            accum_out=zv[:, h, s:s + 1])
                nc.vector.reciprocal(rv[:, 2 * hp:2 * hp + 2, s0:s1],
                                     zv[:, 2 * hp:2 * hp + 2, s0:s1])
                # vp = vh * (1/z), broadcast over d
                vpt = vpp.tile([128, 512], bf16, tag="vp", name="vp_t")
                vpv = vpt[:, :ls * 128].rearrange(
                    "p (s h2 dd) -> p s h2 dd", s=ls, h2=2)
                src = vhv[:, s0:s1, hp * 128:(hp + 1) * 128].rearrange(
                    "p s (h2 dd) -> p s h2 dd", h2=2)
                rtb = rvT[:, s0:s1, 2 * hp:2 * hp + 2].unsqueeze(
                    3).broadcast_to([128, ls, 2, DH])
                nc.gpsimd.tensor_mul(vpv, src, rtb)
                pend[0] = (hp, qi, vpv, ex)
                yield "p3"
        do_av(*pend[0])
        pend[0] = None

        # ---- P4 ----
        for s in range(S):
            yield "p4"
            ps = pq.tile([128, D], f32, tag="pq", name="ps_o")
            for ci in range(C):
                nc.tensor.matmul(
                    ps[:],
                    lhsT=yt[ci][:, s * 128:(s + 1) * 128],
                    rhs=wo[ci][:],
                    start=(ci == 0), stop=(ci == C - 1 and b == 1))
            osb = osbp.tile([128, D], bf16, tag="osb", name="osb")
            if b == 0:
                # P4(b0) executes overlapping P3(b1), where DVE is the
                # binding engine: bo via ones-matmul, copy on ACT
                nc.tensor.matmul(ps[:], lhsT=ones_b[:], rhs=bo_t[0:1, :],
                                 start=False, stop=True)
                nc.scalar.copy(osb[:], ps[:])
            else:
                nc.vector.tensor_add(osb[:], ps[:], bo_t[:])
            eng = nc.sync if s % 2 == 0 else nc.scalar
            eng.dma_start(out_d[b, s], osb[:])

    # ---------------- driver ----------------
    # kron || P1(b0); kron fully drained before P3(b0)'s first stt hits
    # the DVE queue (its reads of tb must queue after all transposes);
    # then P3(b0) || P1(b1) with b1's QK-co_k gated on b0's hp_k scores
    # (qkh slot release order keeps the engine queues acyclic);
    # P3(b1)/P4(b1) emitted only after g0 is fully emitted.
    gk, g0, g1 = kron_gen(), batch_work(0), batch_work(1)
    t0 = next(g0)
    t1 = None
    g1_started = False
    kron_alive = True
    p3_cnt = 0

    def is_p1(t):
        return t is not None and t.startswith("p1")

    while t0 is not None or t1 is not None:
        if t0 is not None:
            t0 = next(g0, None)
            if t0 == "p3":
                p3_cnt += 1
        if kron_alive:
            # ~4 kron units per P1(b0) unit; full drain once g0 leaves P1
            n = 4 if is_p1(t0) else 10 ** 9
            for _ in range(n):
                if next(gk, None) is None:
                    kron_alive = False
                    break
        if not g1_started:
            # start g1 as soon as b0's QK projections are emitted so
            # b1's input DMAs stream during P3(b0), not after it
            if t0 is None or not is_p1(t0) or t0 == "p1v":
                g1_started = True
                t1 = next(g1, None)
        elif t1 is not None:
            if t0 is None:
                # g0 fully emitted: drain g1
                while t1 is not None:
                    t1 = next(g1, None)
            elif is_p1(t1):
                # advance g1 through its P1 only; hold at a gate marker
                # p1g{k} until b0's hp_k scores are emitted; g1's P3
                # waits for g0 to finish entirely
                for _ in range(3):
                    if t1.startswith("p1g"):
                        k = int(t1[3:])
                        if p3_cnt < 8 * k + 5:
                            break
                    t1 = next(g1, None)
                    if t1 is None or not is_p1(t1):
                        break


def build_nc():
    nc = bacc.Bacc("TRN2", target_bir_lowering=False, debug=False,
                   num_devices=NCORES)
    io = {}
    # s-major tokens: [b, d, s, n]
    io["q"] = nc.dram_tensor("q", [B_LOC, D, S, N], bf16, kind="ExternalInput").ap()
    io["k"] = nc.dram_tensor("k", [B_LOC, D, S, N], bf16, kind="ExternalInput").ap()
    io["v"] = nc.dram_tensor("v", [B_LOC, D, S, N], bf16, kind="ExternalInput").ap()
    # ab host layout: [quar, half, (j b2 l)=96, (chl2 hq h2 nq mi n2)=4096]
    io["ab"] = nc.dram_tensor("ab", [4, 2, 96, 4096], bf16,
                              kind="ExternalInput").ap()
    for nm in ("wqT", "wkT", "wvT", "woT"):
        io[nm] = nc.dram_tensor(nm, [D, D], bf16, kind="ExternalInput").ap()
    io["bk"] = nc.dram_tensor("bk", [D], f32, kind="ExternalInput").ap()
    io["bv2"] = nc.dram_tensor("bv2", [128, D], bf16, kind="ExternalInput").ap()
    io["bo2"] = nc.dram_tensor("bo2", [128, D], bf16, kind="ExternalInput").ap()
    io["wdk2"] = nc.dram_tensor("wdk2", [96, 128], bf16, kind="ExternalInput").ap()
    io["out"] = nc.dram_tensor("out", [B_LOC, S, N, D], bf16,
                               kind="ExternalOutput").ap()

    with tile.TileContext(nc) as tc:
        with ExitStack() as ctx:
            emit_kernel(ctx, tc, io)
    nc.compile()
    return nc


def host_prep(Wq, bq, Wk, bk, Wv, bv, Wd, bd, Wo, bo):
    """Pre-transpose weights to bf16; fold qk scale into Wq; drop bq/bd
    (constant along the softmax axis); build the two-batch kron weight."""
    import ml_dtypes
    b16 = ml_dtypes.bfloat16
    scale = (D // H) ** -0.5
    prep = {
        "wqT": np.ascontiguousarray((Wq * scale).T).astype(b16),
        "wkT": np.ascontiguousarray(Wk.T).astype(b16),
        "wvT": np.ascontiguousarray(Wv.T).astype(b16),
        "woT": np.ascontiguousarray(Wo.T).astype(b16),
        "bk": np.asarray(bk, np.float32),
        "bv2": np.ascontiguousarray(
            np.broadcast_to(np.asarray(bv, np.float32), (128, D))).astype(b16),
        "bo2": np.ascontiguousarray(
            np.broadcast_to(np.asarray(bo, np.float32), (128, D))).astype(b16),
    }
    wdk2 = np.zeros((96, 128), np.float32)
    WdT = np.asarray(Wd).T  # [L, S]
    for j in range(4):
        for b2 in range(2):
            r0, c0 = j * 24 + b2 * 12, j * 32 + b2 * 16
            wdk2[r0:r0 + L, c0:c0 + S] = WdT
    prep["wdk2"] = wdk2.astype(b16)
    return prep


_NC_CACHE = None


def run(q, k, v, attn_bias, Wq, bq, Wk, bk, Wv, bv, Wd, bd, Wo, bo,
        trace=False, **trace_kwargs):
    global _NC_CACHE
    import ml_dtypes
    from concourse.bass_utils import run_bass_kernel_spmd

    b16 = ml_dtypes.bfloat16
    if _NC_CACHE is None:
        _NC_CACHE = build_nc()
    nc = _NC_CACHE

    prep = host_prep(Wq, bq, Wk, bk, Wv, bv, Wd, bd, Wo, bo)
    # [B, D, N, S] -> [B, D, S, N]
    q = np.asarray(q).transpose(0, 1, 3, 2).astype(b16)
    k = np.asarray(k).transpose(0, 1, 3, 2).astype(b16)
    v = np.asarray(v).transpose(0, 1, 3, 2).astype(b16)
    ab = np.asarray(attn_bias)
    B = ab.shape[0]

    in_maps = []
    for i in range(NCORES):
        sl = slice(i * B_LOC, (i + 1) * B_LOC)
        # per-core ab: [b2, l, n=(quar half chl2 nq n2), m=(j mi),
        # h=(hq h2)] -> [quar, half, j, b2, l, chl2, hq, h2, nq, mi, n2]
        abc = ab[sl].reshape(2, L, 4, 2, 2, 4, 2, 4, 32, 4, 2)
        abc = abc.transpose(2, 3, 7, 0, 1, 4, 9, 10, 5, 8, 6)
        abc = np.ascontiguousarray(abc).astype(b16).reshape(4, 2, 96, 4096)
        in_maps.append({
            "q": np.ascontiguousarray(q[sl]),
            "k": np.ascontiguousarray(k[sl]),
            "v": np.ascontiguousarray(v[sl]),
            "ab": abc,
            **prep,
        })
    res = run_bass_kernel_spmd(nc, in_maps, list(range(NCORES)), trace=trace,
                               **trace_kwargs)
    out = np.concatenate(
        [res.results[i]["out"].astype(np.float32) for i in range(NCORES)],
        axis=0)
    return out, res


def kernel(**inputs):
    return run(**inputs)[0]


# revision 53
# speedup vs baseline: 1.0457x; 1.0457x over previous
"""Trainium2 Bass kernel v7 for nn_MultiHeadAttention_86079734546451.

Sharding: data-parallel over batch B=16 across 8 cores (2 batches/core).
All weights replicated. No collectives. HW exec ~275us (v2 was 417us).

Design:
 - q/k/v host-transposed to [B, D, S, N] (s-major tokens): every per-s
   slice in scores/V-proj/AV is contiguous (v2 paid ~3x on strided
   s::S LDWEIGHTS/rhs streaming).
 - bias kron packs BOTH batches in one matmul: contraction (j,b2,l)=96
   partitions, block-diag wdk2 outputs (j,b2,s:16) partitions. Halves
   kron MMs, psum-evac copies, and transposes.
 - psum->sbuf kron copy IS the exp: softmax factorized as
   exp(scores+bias) = exp(scores)*exp(bias); ebias moved to m-partition
   layout by a DVE 32x32 stream transpose on an int32 view (n2 pair
   rides inside each i32; the transpose OUT AP scatters (b2,s) outward)
   so each per-(b,h,s) bias slice is a contiguous [128,128].
 - scores exp: ACT reads psum directly per s-quad; the bias multiply is
   a DVE scalar_tensor_tensor per (s,h2) with accum_out -> softmax z
   falls out free (no tensor_reduce, no psum tensor_add).
 - bv/bo folded into psum evacuation (DVE tensor_add with partition-
   replicated bias tiles); output staged bf16; no ones-matmuls.
 - driver: kron||P1(b0) (kron drained before P3's first DVE read of
   tb), P3(b0)||P1(b1) with b1's QK-co_k emission gated on b0's hp_k
   scores (per-name bufs=1 qkh tags make b1 reuse b0's tiles in exact
   release order -> acyclic engine queues), P4(b0)||P3(b1).
 - ebp bufs=5 lets the exp/mul/z chain run a full s-quad ahead of the
   deferred AV matmuls (biggest single win: ~30us).
"""

import sys

sys.path.insert(0, "/opt/trn_rl_repo")

from contextlib import ExitStack

import numpy as np

import concourse.bass as bass
import concourse.mybir as mybir
import concourse.tile as tile
from concourse import bacc

f32 = mybir.dt.float32
bf16 = mybir.dt.bfloat16
i32 = mybir.dt.int32
AF = mybir.ActivationFunctionType
ALU = mybir.AluOpType
AX = mybir.AxisListType

# Problem constants
B_LOC = 2          # batches per core
D = 512
N = 128            # nodes
S = 14             # seq
L = 12
H = 8
DH = 64            # head dim
TOK = N * S        # 1792 tokens per batch, (s, n) order
C = 4              # 128-chunks of D
NCORES = 8

QUADS = [(0, 4), (4, 8), (8, 12), (12, 14)]  # s-blocks


def emit_kernel(ctx: ExitStack, tc: "tile.TileContext", io: dict):
    nc = tc.nc

    q_d, k_d, v_d, ab_d = io["q"], io["k"], io["v"], io["ab"]
    out_d = io["out"]

    # ---------------- pools ----------------
    wpool = ctx.enter_context(tc.tile_pool(name="wpool", bufs=1))
    tbp = ctx.enter_context(tc.tile_pool(name="tbp", bufs=1))
    bsbp = ctx.enter_context(tc.tile_pool(name="bsbp", bufs=2))
    abp = ctx.enter_context(tc.tile_pool(name="abp", bufs=1))
    xin = ctx.enter_context(tc.tile_pool(name="xin", bufs=8))
    qkh = ctx.enter_context(tc.tile_pool(name="qkh", bufs=1))
    vhp = ctx.enter_context(tc.tile_pool(name="vhp", bufs=2))
    ebp = ctx.enter_context(tc.tile_pool(name="ebp", bufs=5))
    ytp = ctx.enter_context(tc.tile_pool(name="ytp", bufs=4))
    vpp = ctx.enter_context(tc.tile_pool(name="vpp", bufs=2))
    zrp = ctx.enter_context(tc.tile_pool(name="zrp", bufs=1))
    osbp = ctx.enter_context(tc.tile_pool(name="osbp", bufs=2))

    pb = ctx.enter_context(tc.tile_pool(name="pb", bufs=2, space="PSUM"))
    pq = ctx.enter_context(tc.tile_pool(name="pq", bufs=2, space="PSUM"))
    scp = ctx.enter_context(tc.tile_pool(name="scp", bufs=2, space="PSUM"))

    # ---------------- weights (once) ----------------
    # wdk2 first (on the empty vector queue) so the kron's first matmul
    # isn't stuck behind 16 weight-tile DMAs; wq tiles next so Q-proj
    # can start as early as possible
    wdk2 = wpool.tile([96, 128], bf16, name="wdk2", tag="wdk2")
    nc.sync.dma_start(wdk2[:], io["wdk2"][:])

    wq, wk, wv, wo = [], [], [], []
    for dst, nm in ((wq, "wqT"), (wk, "wkT"), (wv, "wvT"), (wo, "woT")):
        for c in range(C):
            t = wpool.tile([128, D], bf16, name=f"{nm}{c}", tag=f"{nm}{c}")
            nc.scalar.dma_start(t[:], io[nm][c * 128:(c + 1) * 128, :])
            dst.append(t)

    bk_t = wpool.tile([128, C], f32, name="bk_t", tag="bk_t")
    for c in range(C):
        nc.scalar.dma_start(bk_t[:, c:c + 1],
                            io["bk"][c * 128:(c + 1) * 128].unsqueeze(1))

    bv_t = wpool.tile([128, D], bf16, name="bv_t", tag="bv_t")
    nc.gpsimd.dma_start(bv_t[:], io["bv2"][:])
    bo_t = wpool.tile([128, D], bf16, name="bo_t", tag="bo_t")
    nc.gpsimd.dma_start(bo_t[:], io["bo2"][:])

    # ---------------- shared bias tile (both batches) ----------------
    # tb free layout: (b2:2, s:16, hq:4, h2:2, ch:16, nq:4, n2:2), so the
    # per-(b,hq,h2,s) bias slice [128, n:128] is fully contiguous (the
    # stream-transpose output AP scatters (b2,s) outward; the n2 pair
    # rides inside each int32 element)
    tb = tbp.tile([128, 16 * 2048], bf16, name="tb", tag="tb")
    tb32v = tb[:].bitcast(i32).rearrange(
        "p (b2 s hq h2 ch nq) -> p ch hq h2 nq (b2 s)",
        b2=2, s=16, hq=4, h2=2, ch=16, nq=4)
    tbv = tb[:].rearrange(
        "p (b2 s hq h2 r) -> p b2 hq h2 s r",
        b2=2, s=16, hq=4, h2=2)

    def kron_gen():
        """ebias = exp(Wd-projected attn_bias), transposed to m-partition
        layout, both batches at once."""
        for quar in range(4):
            for half in range(2):
                abt = abp.tile([96, 4096], bf16, tag="abt", name="abt")
                nc.sync.dma_start(abt[:], ab_d[quar, half])
                abtv = abt[:].rearrange(
                    "p (c2 hq r) -> p c2 hq r", c2=2, hq=4)
                for chl in range(2):
                    ch = quar * 4 + half * 2 + chl
                    bsb = bsbp.tile([128, 2048], bf16, tag="bsb", name="bsb")
                    for t in range(2):
                        pbt = pb.tile([128, 1024], f32, tag="pb", name="pbt")
                        for hq2 in range(2):
                            hq = t * 2 + hq2
                            nc.tensor.matmul(
                                pbt[:, hq2 * 512:(hq2 + 1) * 512],
                                lhsT=wdk2[:],
                                rhs=abtv[:, chl, hq, :],
                                start=True, stop=True)
                        nc.scalar.activation(
                            bsb[:, t * 1024:(t + 1) * 1024], pbt[:], AF.Exp)
                        yield "kron"
                    nc.vector.transpose(tb32v[:, ch], bsb[:].bitcast(i32))
                    yield "kron"

    # ---------------- per-batch body ----------------
    def batch_work(b):
        def load_x(src_d):
            xs = []
            for ci in range(C):
                x_c = xin.tile([128, TOK], bf16, tag="xin", name=f"x{ci}")
                nc.gpsimd.dma_start(
                    x_c[:],
                    src_d[b, ci * 128:(ci + 1) * 128].rearrange(
                        "p s n -> p (s n)"))
                xs.append(x_c)
            return xs

        def p1_co(xs, wts, co, dst_list, with_bias, tg):
            # per-name tag, bufs=1: b1's gen reuses exactly b0's slot,
            # which frees when b0's P3 hp=co scores have read it
            h_c = qkh.tile([128, TOK], bf16, tag=f"h{tg}{co}", name=f"h{tg}{co}")
            for tbk in range(4):
                ps = pq.tile([128, 512], f32, tag="pq", name="ps_qk")
                for ci in range(C):
                    nc.tensor.matmul(
                        ps[:, :448],
                        lhsT=wts[ci][:, co * 128:(co + 1) * 128],
                        rhs=xs[ci][:, tbk * 448:(tbk + 1) * 448],
                        start=(ci == 0), stop=(ci == C - 1))
                if with_bias:
                    nc.scalar.activation(h_c[:, tbk * 448:(tbk + 1) * 448],
                                         ps[:, :448], AF.Identity,
                                         bias=bk_t[:, co:co + 1], scale=1.0)
                else:
                    # Q copies on DVE to balance ACT (kron exp runs there)
                    nc.vector.tensor_copy(h_c[:, tbk * 448:(tbk + 1) * 448],
                                          ps[:, :448])
            dst_list.append(h_c)

        # ---- P1 ----
        qh, kh = [], []
        yield "p1"
        xq = load_x(q_d)
        xk = load_x(k_d)
        for co in range(C):
            # gate marker: for b1 the driver holds here until b0's P3
            # hp=co scores are emitted (qkh slot release order)
            yield f"p1g{co}"
            p1_co(xq, wq, co, qh, False, "q")
            p1_co(xk, wk, co, kh, True, "k")
        yield "p1v"
        xv = load_x(v_d)
        vh = vhp.tile([128, S * D], bf16, tag="vh", name="vh")
        for s in range(S):
            ps = pq.tile([128, D], f32, tag="pq", name="ps_v")
            for ci in range(C):
                nc.tensor.matmul(
                    ps[:],
                    lhsT=xv[ci][:, s * N:(s + 1) * N],
                    rhs=wv[ci][:],
                    start=(ci == 0), stop=(ci == C - 1))
            nc.vector.tensor_add(vh[:, s * D:(s + 1) * D], ps[:], bv_t[:])
            if s % 2:
                yield "p1v"

        # ---- P3 ----
        z_t = zrp.tile([128, 128], f32, tag="z", name="z_t")
        rt_t = zrp.tile([128, 128], f32, tag="r", name="rt_t")
        zv = z_t[:].rearrange("p (h s) -> p h s", h=H)
        rv = rt_t[:].rearrange("p (h s) -> p h s", h=H)
        rvT = rt_t[:].rearrange("p (h s) -> p s h", h=H)
        vhv = vh[:].rearrange("p (s d) -> p s d", s=S)
        yt = [ytp.tile([128, TOK], bf16, tag="ytp", name=f"yt{c}")
              for c in range(C)]

        pend = [None]  # deferred AV work (one-quad software pipeline)

        def do_av(hp, qi, vpv, ex):
            s0, s1 = QUADS[qi]
            ls = s1 - s0
            av = pb.tile([128, 512], f32, tag="pb", name="av_t")
            for si in range(ls):
                for h2 in range(2):
                    nc.tensor.matmul(
                        av[h2 * DH:(h2 + 1) * DH, si * 128:(si + 1) * 128],
                        lhsT=vpv[:, si, h2, :],
                        rhs=ex[h2][:, si * 128:(si + 1) * 128],
                        start=True, stop=True)
            nc.scalar.copy(yt[hp][:, s0 * 128:s1 * 128], av[:, :ls * 128])

        for hp in range(C):
            for qi, (s0, s1) in enumerate(QUADS):
                ls = s1 - s0
                # scores for this quad
                sct = [scp.tile([128, 512], f32, tag="sc", name=f"sc{h2}")
                       for h2 in range(2)]
                for si in range(ls):
                    s = s0 + si
                    for h2 in range(2):
                        hb = h2 * DH
                        nc.tensor.matmul(
                            sct[h2][:, si * 128:(si + 1) * 128],
                            lhsT=kh[hp][hb:hb + DH, s * N:(s + 1) * N],
                            rhs=qh[hp][hb:hb + DH, s * N:(s + 1) * N],
                            start=True, stop=True)
                yield "p3"
                # deferred AV of previous quad (PE work lands after the
                # interleaved P1 units gave the chain time to finish)
                if pend[0] is not None:
                    do_av(*pend[0])
                    pend[0] = None
                # exp(scores) from psum; bias-mul with accum -> z
                ex = [ebp.tile([128, 512], bf16, tag="eb", name=f"eb{h2}")
                      for h2 in range(2)]
                for h2 in range(2):
                    nc.scalar.activation(ex[h2][:, :ls * 128],
                                         sct[h2][:, :ls * 128], AF.Exp)
                for h2 in range(2):
                    h = hp * 2 + h2
                    for si in range(ls):
                        s = s0 + si
                        esl = ex[h2][:, si * 128:(si + 1) * 128]
                        nc.vector.scalar_tensor_tensor(
                            esl, esl, 1.0, tbv[:, b, hp, h2, s],
                            op0=ALU.mult, op1=ALU.mult,
                            accum_out=zv[:, h, s:s + 1])
                nc.vector.reciprocal(rv[:, 2 * hp:2 * hp + 2, s0:s1],
                                     zv[:, 2 * hp:2 * hp + 2, s0:s1])
                # vp = vh * (1/z), broadcast over d
                vpt = vpp.tile([128, 512], bf16, tag="vp", name="vp_t")
                vpv = vpt[:, :ls * 128].rearrange(
                    "p (s h2 dd) -> p s h2 dd", s=ls, h2=2)
                src = vhv[:, s0:s1, hp * 128:(hp + 1) * 128].rearrange(
                    "p s (h2 dd) -> p s h2 dd", h2=2)
                rtb = rvT[:, s0:s1, 2 * hp:2 * hp + 2].unsqueeze(
                    3).broadcast_to([128, ls, 2, DH])
                nc.gpsimd.tensor_mul(vpv, src, rtb)
                pend[0] = (hp, qi, vpv, ex)
                yield "p3"
        do_av(*pend[0])
        pend[0] = None

        # ---- P4 ----
        for s in range(S):
            yield "p4"
            ps = pq.tile([128, D], f32, tag="pq", name="ps_o")
            for ci in range(C):
                nc.tensor.matmul(
                    ps[:],
                    lhsT=yt[ci][:, s * 128:(s + 1) * 128],
                    rhs=wo[ci][:],
                    start=(ci == 0), stop=(ci == C - 1))
            osb = osbp.tile([128, D], bf16, tag="osb", name="osb")
            nc.vector.tensor_add(osb[:], ps[:], bo_t[:])
            eng = nc.sync if s % 2 == 0 else nc.scalar
            eng.dma_start(out_d[b, s], osb[:])

    # ---------------- driver ----------------
    # kron || P1(b0); kron fully drained before P3(b0)'s first stt hits
    # the DVE queue (its reads of tb must queue after all transposes);
    # then P3(b0) || P1(b1) with b1's QK-co_k gated on b0's hp_k scores
    # (qkh slot release order keeps the engine queues acyclic);
    # P3(b1)/P4(b1) emitted only after g0 is fully emitted.
    gk, g0, g1 = kron_gen(), batch_work(0), batch_work(1)
    t0 = next(g0)
    t1 = None
    g1_started = False
    kron_alive = True
    p3_cnt = 0

    def is_p1(t):
        return t is not None and t.startswith("p1")

    while t0 is not None or t1 is not None:
        if t0 is not None:
            t0 = next(g0, None)
            if t0 == "p3":
                p3_cnt += 1
        if kron_alive:
            # ~4 kron units per P1(b0) unit; full drain once g0 leaves P1
            n = 4 if is_p1(t0) else 10 ** 9
            for _ in range(n):
                if next(gk, None) is None:
                    kron_alive = False
                    break
        if not g1_started:
            # start g1 as soon as b0's QK projections are emitted so
            # b1's input DMAs stream during P3(b0), not after it
            if t0 is None or not is_p1(t0) or t0 == "p1v":
                g1_started = True
                t1 = next(g1, None)
        elif t1 is not None:
            if t0 is None:
                # g0 fully emitted: drain g1
                while t1 is not None:
                    t1 = next(g1, None)
            elif is_p1(t1):
                # advance g1 through its P1 only; hold at a gate marker
                # p1g{k} until b0's hp_k scores are emitted; g1's P3
                # waits for g0 to finish entirely
                for _ in range(3):
                    if t1.startswith("p1g"):
                        k = int(t1[3:])
                        if p3_cnt < 8 * k + 5:
                            break
                    t1 = next(g1, None)
                    if t1 is None or not is_p1(t1):
                        break


def build_nc():
    nc = bacc.Bacc("TRN2", target_bir_lowering=False, debug=False,
                   num_devices=NCORES)
    io = {}
    # s-major tokens: [b, d, s, n]
    io["q"] = nc.dram_tensor("q", [B_LOC, D, S, N], bf16, kind="ExternalInput").ap()
    io["k"] = nc.dram_tensor("k", [B_LOC, D, S, N], bf16, kind="ExternalInput").ap()
    io["v"] = nc.dram_tensor("v", [B_LOC, D, S, N], bf16, kind="ExternalInput").ap()
    # ab host layout: [quar, half, (j b2 l)=96, (chl2 hq h2 nq mi n2)=4096]
    io["ab"] = nc.dram_tensor("ab", [4, 2, 96, 4096], bf16,
                              kind="ExternalInput").ap()
    for nm in ("wqT", "wkT", "wvT", "woT"):
        io[nm] = nc.dram_tensor(nm, [D, D], bf16, kind="ExternalInput").ap()
    io["bk"] = nc.dram_tensor("bk", [D], f32, kind="ExternalInput").ap()
    io["bv2"] = nc.dram_tensor("bv2", [128, D], bf16, kind="ExternalInput").ap()
    io["bo2"] = nc.dram_tensor("bo2", [128, D], bf16, kind="ExternalInput").ap()
    io["wdk2"] = nc.dram_tensor("wdk2", [96, 128], bf16, kind="ExternalInput").ap()
    io["out"] = nc.dram_tensor("out", [B_LOC, S, N, D], bf16,
                               kind="ExternalOutput").ap()

    with tile.TileContext(nc) as tc:
        with ExitStack() as ctx:
            emit_kernel(ctx, tc, io)
    nc.compile()
    return nc


def host_prep(Wq, bq, Wk, bk, Wv, bv, Wd, bd, Wo, bo):
    """Pre-transpose weights to bf16; fold qk scale into Wq; drop bq/bd
    (constant along the softmax axis); build the two-batch kron weight."""
    import ml_dtypes
    b16 = ml_dtypes.bfloat16
    scale = (D // H) ** -0.5
    prep = {
        "wqT": np.ascontiguousarray((Wq * scale).T).astype(b16),
        "wkT": np.ascontiguousarray(Wk.T).astype(b16),
        "wvT": np.ascontiguousarray(Wv.T).astype(b16),
        "woT": np.ascontiguousarray(Wo.T).astype(b16),
        "bk": np.asarray(bk, np.float32),
        "bv2": np.ascontiguousarray(
            np.broadcast_to(np.asarray(bv, np.float32), (128, D))).astype(b16),
        "bo2": np.ascontiguousarray(
            np.broadcast_to(np.asarray(bo, np.float32), (128, D))).astype(b16),
    }
    wdk2 = np.zeros((96, 128), np.float32)
    WdT = np.asarray(Wd).T  # [L, S]
    for j in range(4):
        for b2 in range(2):
            r0, c0 = j * 24 + b2 * 12, j * 32 + b2 * 16
            wdk2[r0:r0 + L, c0:c0 + S] = WdT
    prep["wdk2"] = wdk2.astype(b16)
    return prep


_NC_CACHE = None


def run(q, k, v, attn_bias, Wq, bq, Wk, bk, Wv, bv, Wd, bd, Wo, bo,
        trace=False, **trace_kwargs):
    global _NC_CACHE
    import ml_dtypes
    from concourse.bass_utils import run_bass_kernel_spmd

    b16 = ml_dtypes.bfloat16
    if _NC_CACHE is None:
        _NC_CACHE = build_nc()
    nc = _NC_CACHE

    prep = host_prep(Wq, bq, Wk, bk, Wv, bv, Wd, bd, Wo, bo)
    # [B, D, N, S] -> [B, D, S, N]
    q = np.asarray(q).transpose(0, 1, 3, 2).astype(b16)
    k = np.asarray(k).transpose(0, 1, 3, 2).astype(b16)
    v = np.asarray(v).transpose(0, 1, 3, 2).astype(b16)
    ab = np.asarray(attn_bias)
    B = ab.shape[0]

    in_maps = []
    for i in range(NCORES):
        sl = slice(i * B_LOC, (i + 1) * B_LOC)
        # per-core ab: [b2, l, n=(quar half chl2 nq n2), m=(j mi),
        # h=(hq h2)] -> [quar, half, j, b2, l, chl2, hq, h2, nq, mi, n2]
        abc = ab[sl].reshape(2, L, 4, 2, 2, 4, 2, 4, 32, 4, 2)
        abc = abc.transpose(2, 3, 7, 0, 1, 4, 9, 10, 5, 8, 6)
        abc = np.ascontiguousarray(abc).astype(b16).reshape(4, 2, 96, 4096)
        in_maps.append({
            "q": np.ascontiguousarray(q[sl]),
            "k": np.ascontiguousarray(k[sl]),
            "v": np.ascontiguousarray(v[sl]),
            "ab": abc,
            **prep,
        })
    res = run_bass_kernel_spmd(nc, in_maps, list(range(NCORES)), trace=trace,
                               **trace_kwargs)
    out = np.concatenate(
        [res.results[i]["out"].astype(np.float32) for i in range(NCORES)],
        axis=0)
    return out, res


def kernel(**inputs):
    return run(**inputs)[0]
